# revision 1
# baseline (speedup 1.0000x reference)
"""Trainium2 Bass kernel for CapsuleLayer dynamic routing (v2).

Problem: x [64, 2048, 16], W [1, 2048, 32, 32, 16] ->
  u_hat = einsum('bik,ijdk->bijd', x, W[0])           [B, N_in, N_out, D_out]
  3 rounds of routing (softmax over j, weighted sum over i, squash),
  returns v [64, 32, 32].

Sharding: N_in (2048) split over 8 cores, 256 input capsules each; per-round
partial weighted sums are AllReduced; squash/softmax replicated.

Per-core design (derived from a perfetto-trace cost model of the v1 kernel,
which was DVE-bound at 81% occupancy):
  * Batch processed in 2 chunks of 32 (u_hat chunk = 16.8MB bf16 in SBUF).
  * U layout [p=(q4, b32); (blk32, g2 2, d32, j32)] with j INNERMOST:
    both big per-round multiplies (U*v-broadcast and U*c-broadcast) hit the
    DVE 2x_1P mode (broadcast strides live on non-innermost dims).
  * All big reductions are TT-add TREES at 2x instead of native
    tensor_reduce (which is capped at 1x).
  * Phase 1 computes u_hat with 8-way 32x32 PE-array tile packing
    (row bands (g2,gp) x col bands (gp,h)), sharing one W stream per blk
    with the K=128 full-array s0 matmuls (t=0 shortcut, keeps HAM warm).
  * PSUM->SBUF drains split between DVE (CAST 2x) and ACT.
  * Final (sec, q) reduction of weighted-sum partials via a ones-matmul on
    the PE with psum accumulation.
  * squash uses only exp/ln (one ACT table set): rsqrt(x) = exp(-0.5*ln(x)).
"""
import sys

sys.path.insert(0, '/opt/trn_rl_repo')

import numpy as np

import concourse.bass as bass
import concourse.mybir as mybir
from concourse import bass_utils, tile

# ---------------------------------------------------------------- constants
N_CORES = 8
B = 64
N_IN = 2048
D_IN = 16
N_OUT = 32
D_OUT = 32
ROUTINGS = 3
EPS = 1e-9

I_LOC = N_IN // N_CORES          # 256 local capsules
NBLK = I_LOC // 8                # 32 blocks of 8 capsules
BC = 32                          # batch chunk
NCHUNK = B // BC                 # 2
JD = N_OUT * D_OUT               # 1024 (d,j) values per capsule
NSEC = 8                         # routing sections per chunk-round
SEC_BLKS = NBLK // NSEC          # 4 blocks per section
SEC_BH = SEC_BLKS * 2            # 8 (blk, g2) groups per section
SEC_COLS = SEC_BH * JD           # 8192 U-columns per section

f32 = mybir.dt.float32
bf16 = mybir.dt.bfloat16

_MAX_WAITS = 1
_carrier = [0]


def _patch_tile():
    """Work around this walrus build rejecting >1 sync wait per instruction."""
    import concourse.mybir as _mybir
    from concourse import tile as _tile
    from concourse.tile import TileContext as _TC

    def _drain_and_barrier(self, tick_clock, wait_clock):
        ScopedClock = _tile.ScopedClock
        probe = self.nc.sync.nop(nofuse=True)
        wait_clock.add_sem_waits(
            probe.ins, ScopedClock({None: tick_clock.global_clock})
        )
        si = probe.ins.sync_info
        waits = list(si.on_wait)
        probe.ins.sync_info = _mybir.SyncInfo(
            on_wait=waits[:1], on_update=list(si.on_update)
        )
        for w in waits[1:]:
            carrier = self.nc.sync.nop(nofuse=True)
            carrier.ins.sync_info = _mybir.SyncInfo(on_wait=[w], on_update=[])
        self.nc.sync.drain()
        self.nc.all_engine_barrier()
        assert self.sems is not None
        popped = self.nc._tile_sem_poison_stack.pop()
        assert popped is self._sem_poison
        self.nc.clear_and_free_semaphores(list(self.sems.allocated().values()))
        self.nc.all_engine_barrier()

    _TC._drain_and_barrier = _drain_and_barrier

    try:
        from concourse import tile_utils
        tile_utils.max_sbuf_usage = 208 * 1024
    except Exception:
        pass


def _fix_sync_waits(nc, max_waits=_MAX_WAITS):
    n_fixed = 0
    for func in nc.m.functions:
        for bb in func.blocks:
            insts = list(bb.instructions)
            new_list = []
            changed = False
            for inst in insts:
                si = getattr(inst, "sync_info", None)
                waits = list(si.on_wait) if si is not None else []
                if len(waits) > max_waits:
                    keep = waits[: max_waits - 1] if max_waits > 1 else []
                    hoist = waits[len(keep):-1]
                    tail = [waits[-1]]
                    for w in hoist:
                        _carrier[0] += 1
                        nop = mybir.InstNoOp(
                            name=f"syncfix-{_carrier[0]}", engine=inst.engine
                        )
                        nop.sync_info = mybir.SyncInfo(on_wait=[w], on_update=[])
                        new_list.append(nop)
                    inst.sync_info = mybir.SyncInfo(
                        on_wait=keep + tail, on_update=list(si.on_update)
                    )
                    changed = True
                    n_fixed += 1
                new_list.append(inst)
            if changed:
                bb.instructions = new_list
    return n_fixed


# ---------------------------------------------------------------- program
def _build_program():
    _patch_tile()
    nc = bass.Bass(trn_type="TRN2", num_devices=N_CORES)

    wt_in = nc.dram_tensor("wt", [128, NBLK * JD], bf16, kind="ExternalInput")
    xz_in = nc.dram_tensor("xz", [128, NBLK * 2 * B], bf16, kind="ExternalInput")
    xd_in = nc.dram_tensor("xd", [128, NBLK * B], bf16, kind="ExternalInput")
    ones_in = nc.dram_tensor("ones32", [128, BC], bf16, kind="ExternalInput")
    v_out = nc.dram_tensor("v", [B, JD], f32, kind="ExternalOutput")

    AluOp = mybir.AluOpType
    Act = mybir.ActivationFunctionType
    Axis = mybir.AxisListType
    rg = [list(range(N_CORES))]

    from contextlib import ExitStack
    with tile.TileContext(nc, num_cores=N_CORES) as tc, ExitStack() as es:
        cpool = es.enter_context(tc.tile_pool(name="const", bufs=1))
        wpool = es.enter_context(tc.tile_pool(name="wstream", bufs=5))
        upool = es.enter_context(tc.tile_pool(name="ubuf", bufs=1))
        scpool = es.enter_context(tc.tile_pool(name="scratch", bufs=1))
        smpool = es.enter_context(tc.tile_pool(name="small", bufs=1))
        ps1pool = es.enter_context(tc.tile_pool(name="psph1", bufs=2, space="PSUM"))
        ps0pool = es.enter_context(tc.tile_pool(name="psums0", bufs=1, space="PSUM"))
        psrpool = es.enter_context(tc.tile_pool(name="psred", bufs=1, space="PSUM"))
        dpool = es.enter_context(tc.tile_pool(name="dram", bufs=1, space="DRAM"))

        # ---- constants / inputs resident in SBUF
        epsc = cpool.tile([128, 1], f32, tag="epsc")
        nc.vector.memset(epsc[:], EPS)
        xz = cpool.tile([128, NBLK * 2 * B], bf16)
        xd = cpool.tile([128, NBLK * B], bf16)
        ones32 = cpool.tile([128, BC], bf16)
        nc.sync.dma_start(xd[:], xd_in[:])
        nc.sync.dma_start(ones32[:], ones_in[:])

        # ---- big buffers
        U = upool.tile([128, NBLK * 2 * JD], bf16, tag="U")       # 128KB/p
        prodb = scpool.tile([128, SEC_COLS], bf16, tag="prod")    # 16KB/p
        treeb = scpool.tile([128, SEC_COLS // 2], bf16, tag="tree")  # 8KB/p
        algA = smpool.tile([128, NBLK * 2 * N_OUT], f32, tag="algA")  # 8KB/p
        cbuf = smpool.tile([128, NBLK * 2 * N_OUT], bf16, tag="c")
        Zt = smpool.tile([128, NBLK * 2], f32, tag="Z")
        Zr = smpool.tile([128, NBLK * 2], f32, tag="Zr")
        s_rep = smpool.tile([128, JD], f32, tag="srep")
        s2 = smpool.tile([128, N_OUT], f32, tag="s2")
        lns = smpool.tile([128, N_OUT], f32, tag="lns")
        rsq = smpool.tile([128, N_OUT], f32, tag="rsq")
        den = smpool.tile([128, N_OUT], f32, tag="den")
        rinv = smpool.tile([128, N_OUT], f32, tag="rinv")
        fsc = smpool.tile([128, N_OUT], f32, tag="fsc")
        ibuf = smpool.tile([128, N_OUT], mybir.dt.int32, tag="ibuf")
        v_bf = smpool.tile([128, JD], bf16, tag="vbf")
        s_sb = smpool.tile([BC, JD], f32, tag="ssb")
        s0_sb = smpool.tile([B, JD], f32, tag="s0sb")
        v_fin = smpool.tile([BC, JD], f32, tag="vfin")

        # aliased scratch views (prodb/treeb free at those times)
        ebuf = prodb[:, :4096].bitcast(f32)          # [128, 2048] exp(logits)
        p2 = prodb[:, 4096:6144].bitcast(f32)        # [128, 1024] squash s^2
        tsec = treeb[:, 2048:2560].bitcast(f32)      # [128, 256] t=2 A-slice

        ps_s0 = ps0pool.tile([B, JD], f32, tag="pss0")

        # AR dram staging
        ar0_in = dpool.tile([B, JD], f32, tag="ar0i")
        ar0_out = dpool.tile([B, JD], f32, tag="ar0o")
        ar_bufs = {}
        for q in range(NCHUNK):
            for t in (1, 2):
                ar_bufs[(q, t)] = (
                    dpool.tile([BC, JD], f32, name=f"ari{q}{t}"),
                    dpool.tile([BC, JD], f32, name=f"aro{q}{t}"),
                )

        # ------------------------------------------------------ phase 1
        def warmup():
            """~6us of back-to-back full-array matmuls to flip the PE HAM
            clock gate to 8/8 before a phase-1 burst (output never read)."""
            ps = ps1pool.tile([128, JD], f32, tag="ps1", name="warm")
            for i in range(14):
                nc.tensor.matmul(
                    ps[0:64, 0:512], xd[:, 0:B], xz[:, 0:512],
                    start=True, stop=True,
                )

        def s0_burst():
            """t=0 shortcut: s0 = (1/32) sum_i u_hat, full batch, K=128
            full-array matmuls, own W stream; runs before phase 1 so the
            AllReduce of s0 completes early (also warms the PE HAM)."""
            for blk in range(NBLK):
                w = wpool.tile([128, JD], bf16, tag="w", name="w0")
                nc.sync.dma_start(w[:], wt_in[:, blk * JD:(blk + 1) * JD])
                for half in range(2):
                    nc.tensor.matmul(
                        ps_s0[:, half * 512:(half + 1) * 512],
                        xd[:, blk * B:(blk + 1) * B],
                        w[:, half * 512:(half + 1) * 512],
                        start=(blk == 0), stop=(blk == NBLK - 1),
                    )

        def phase1(q, agree_lag=None, pre_agree=None, dve_drain_mod=0,
                   vsrc=None):
            """u_hat for batch chunk q into U.  If agree_lag is not None,
            the round-1 agreement sections are emitted inline, lagged by
            `agree_lag` phase-1 sections, to fill the DVE idle window while
            the PE streams the tiled matmuls.  pre_agree() is emitted right
            before the first inlined agreement section.  A drain goes to the
            DVE when (idx % 4) < dve_drain_mod, else to ACT."""
            emitted_pre = [False]
            for blk in range(NBLK):
                w = wpool.tile([128, JD], bf16, tag="w")
                nc.sync.dma_start(w[:], wt_in[:, blk * JD:(blk + 1) * JD])
                for g2 in range(2):
                    ps = ps1pool.tile([128, JD], f32, tag="ps1")
                    for gp in range(2):
                        r = g2 * 2 + gp
                        for h in range(2):
                            m = gp * 2 + h
                            lhs = xz[32 * r:32 * r + 32,
                                     blk * 2 * B + h * B + q * BC:
                                     blk * 2 * B + h * B + q * BC + BC]
                            for half in range(2):
                                nc.tensor.matmul(
                                    ps[32 * m:32 * m + 32,
                                       half * 512:(half + 1) * 512],
                                    lhs,
                                    w[32 * r:32 * r + 32,
                                      half * 512:(half + 1) * 512],
                                    start=True, stop=True,
                                    tile_position=(32 * r, 32 * m),
                                )
                    dst = U[:, blk * 2 * JD + g2 * JD:
                            blk * 2 * JD + (g2 + 1) * JD]
                    if (blk * 2 + g2) % 4 < dve_drain_mod:
                        nc.vector.tensor_copy(dst, ps[:])
                    else:
                        nc.scalar.copy(dst, ps[:])
                if agree_lag is not None and (blk + 1) % SEC_BLKS == 0:
                    p1sec = (blk + 1) // SEC_BLKS - 1
                    asec = p1sec - agree_lag
                    if asec >= 0:
                        if not emitted_pre[0]:
                            emitted_pre[0] = True
                            if pre_agree is not None:
                                pre_agree()
                        agreement_section(1, asec, vsrc=vsrc)
            if agree_lag is not None:
                for asec in range(NSEC - agree_lag, NSEC):
                    agreement_section(1, asec, vsrc=vsrc)

        # ------------------------------------------------------ routing ops
        def agreement_section(t, sec, vsrc=None):
                Us = U[:, sec * SEC_COLS:(sec + 1) * SEC_COLS].rearrange(
                    "p (bh d j) -> p bh d j", bh=SEC_BH, d=D_OUT, j=N_OUT)
                P = prodb[:].rearrange(
                    "p (bh d j) -> p bh d j", bh=SEC_BH, d=D_OUT, j=N_OUT)
                vs = v_bf if vsrc is None else vsrc
                v4 = (vs[:]
                      .rearrange("p (d j) -> p d j", d=D_OUT, j=N_OUT)
                      .unsqueeze(1)
                      .to_broadcast((128, SEC_BH, D_OUT, N_OUT)))
                nc.vector.tensor_tensor(P, Us, v4, AluOp.mult)   # 2x
                # d-tree: 32 -> 16 -> 8 -> 4 -> 2 -> 1
                T1 = treeb[:, :4096].rearrange(
                    "p (bh d j) -> p bh d j", bh=SEC_BH, d=16, j=N_OUT)
                nc.vector.tensor_tensor(
                    T1, P[:, :, 0:16, :], P[:, :, 16:32, :], AluOp.add)
                T2 = prodb[:, :2048].rearrange(
                    "p (bh d j) -> p bh d j", bh=SEC_BH, d=8, j=N_OUT)
                nc.vector.tensor_tensor(
                    T2, T1[:, :, 0:8, :], T1[:, :, 8:16, :], AluOp.add)
                T3 = treeb[:, :1024].rearrange(
                    "p (bh d j) -> p bh d j", bh=SEC_BH, d=4, j=N_OUT)
                nc.vector.tensor_tensor(
                    T3, T2[:, :, 0:4, :], T2[:, :, 4:8, :], AluOp.add)
                T4 = prodb[:, 2048:2560].rearrange(
                    "p (bh d j) -> p bh d j", bh=SEC_BH, d=2, j=N_OUT)
                nc.vector.tensor_tensor(
                    T4, T3[:, :, 0:2, :], T3[:, :, 2:4, :], AluOp.add)
                a_cols = SEC_BH * N_OUT            # 256 logit cols / section
                if t == 1:
                    T5 = algA[:, sec * a_cols:(sec + 1) * a_cols].rearrange(
                        "p (bh d j) -> p bh d j", bh=SEC_BH, d=1, j=N_OUT)
                    nc.vector.tensor_tensor(
                        T5, T4[:, :, 0:1, :], T4[:, :, 1:2, :], AluOp.add)
                else:
                    T5 = tsec.rearrange(
                        "p (bh d j) -> p bh d j", bh=SEC_BH, d=1, j=N_OUT)
                    nc.vector.tensor_tensor(
                        T5, T4[:, :, 0:1, :], T4[:, :, 1:2, :], AluOp.add)
                    asl = algA[:, sec * a_cols:(sec + 1) * a_cols]
                    nc.vector.tensor_add(asl, asl, tsec)

        def softmax():
            nc.scalar.activation(ebuf, algA[:], Act.Exp)
            e3 = ebuf.rearrange("p (bh j) -> p bh j", bh=NBLK * 2, j=N_OUT)
            nc.vector.reduce_sum(Zt[:], e3, axis=Axis.X)
            nc.vector.reciprocal(Zr[:], Zt[:])
            c3 = cbuf[:].rearrange("p (bh j) -> p bh j", bh=NBLK * 2, j=N_OUT)
            zr3 = Zr[:].unsqueeze(2).to_broadcast((128, NBLK * 2, N_OUT))
            nc.vector.tensor_tensor(c3, e3, zr3, AluOp.mult)

        def weighted_sum(q, t):
            """s_partial = sum_i c * U.  The DVE only computes the c*U
            product (2x); ALL the summation (64 bh column-groups AND the 4
            partition groups) happens on the PE as one 128-matmul psum
            accumulation chain through the ones matrix."""
            ps_red = psrpool.tile([BC, JD], f32, tag="psred")
            NSW = 16                       # 2-blk product sections
            SW_BH = NBLK * 2 // NSW        # 4 (blk,g2) groups per section
            SW_COLS = SW_BH * JD           # 4096
            slots = [prodb[:, 0:SW_COLS], prodb[:, SW_COLS:2 * SW_COLS],
                     treeb[:, 0:SW_COLS]]
            for sec in range(NSW):
                slot = slots[sec % 3]
                P = slot.rearrange(
                    "p (bh d j) -> p bh d j", bh=SW_BH, d=D_OUT, j=N_OUT)
                Us = U[:, sec * SW_COLS:(sec + 1) * SW_COLS].rearrange(
                    "p (bh d j) -> p bh d j", bh=SW_BH, d=D_OUT, j=N_OUT)
                a_cols = SW_BH * N_OUT
                c4 = (cbuf[:, sec * a_cols:(sec + 1) * a_cols]
                      .rearrange("p (bh j) -> p bh j", bh=SW_BH, j=N_OUT)
                      .unsqueeze(2)
                      .to_broadcast((128, SW_BH, D_OUT, N_OUT)))
                nc.vector.tensor_tensor(P, Us, c4, AluOp.mult)   # 2x
                for bh in range(SW_BH):
                    for half in range(2):
                        nc.tensor.matmul(
                            ps_red[:, half * 512:(half + 1) * 512],
                            ones32[:],
                            slot[:, bh * JD + half * 512:
                                 bh * JD + half * 512 + 512],
                            start=(sec == 0 and bh == 0),
                            stop=(sec == NSW - 1 and bh == SW_BH - 1),
                        )
            nc.scalar.copy(s_sb[:], ps_red[:])
            ar_in, ar_out = ar_bufs[(q, t)]
            nc.gpsimd.dma_start(ar_in[:], s_sb[:])
            nc.gpsimd.collective_compute(
                "AllReduce", AluOp.add, replica_groups=rg,
                ins=[ar_in.opt()], outs=[ar_out.opt()],
            )
            return ar_out

        def squash(ar_src, row_off, final, q, dma_eng=None, vdst=None):
            """v = squash(s), DVE-only (no ACT/Sync queue head-blocking):
            rsqrt via fast-inverse-sqrt bit trick + 2 Newton steps.
            dma_eng picks the replicate-DMA queue: the gpsimd queue blocks on
            any collective emitted before it, so only the chunk-0 inline
            squash (whose sync queue is full of w-dmas at that point) uses
            gpsimd; everywhere else sync is free."""
            eng = nc.gpsimd if dma_eng == "g" else nc.sync
            for g in range(4):
                eng.dma_start(
                    s_rep[32 * g:32 * g + 32, :],
                    ar_src[row_off:row_off + BC, :],
                )
            nc.vector.tensor_mul(p2, s_rep[:], s_rep[:])
            p3 = p2.rearrange("p (d j) -> p j d", d=D_OUT, j=N_OUT)
            nc.vector.reduce_sum(s2[:], p3, axis=Axis.X)
            nc.vector.tensor_scalar_add(den[:], s2[:], 1.0 + EPS)
            nc.vector.tensor_scalar_add(lns[:], s2[:], EPS)   # x = s2+eps
            ii = lns[:].bitcast(mybir.dt.int32)
            # y0 = bits(0x5f3759df - (bits(x) >> 1))
            nc.vector.tensor_scalar(
                ibuf[:], ii, 1, None,
                mybir.AluOpType.logical_shift_right)
            nc.vector.tensor_scalar(
                ibuf[:], ibuf[:], 0x5F3759DF, -1,
                mybir.AluOpType.subtract, mybir.AluOpType.mult)
            y0 = ibuf[:].bitcast(f32)
            # two Newton steps: y <- y*(1.5 - 0.5*x*y^2)
            nc.vector.tensor_mul(rsq[:], y0, y0)
            nc.vector.tensor_mul(rsq[:], rsq[:], lns[:])
            nc.vector.tensor_scalar(
                rsq[:], rsq[:], -0.5, 1.5,
                mybir.AluOpType.mult, mybir.AluOpType.add)
            nc.vector.tensor_mul(rsq[:], rsq[:], y0)
            nc.vector.tensor_mul(fsc[:], rsq[:], rsq[:])
            nc.vector.tensor_mul(fsc[:], fsc[:], lns[:])
            nc.vector.tensor_scalar(
                fsc[:], fsc[:], -0.5, 1.5,
                mybir.AluOpType.mult, mybir.AluOpType.add)
            nc.vector.tensor_mul(rsq[:], rsq[:], fsc[:])
            nc.vector.reciprocal(rinv[:], den[:])
            nc.vector.tensor_mul(fsc[:], rsq[:], rinv[:])
            nc.vector.tensor_mul(fsc[:], fsc[:], s2[:])
            s3 = s_rep[:].rearrange("p (d j) -> p d j", d=D_OUT, j=N_OUT)
            if not final:
                vd = v_bf if vdst is None else vdst
                f3 = fsc[:].unsqueeze(1).to_broadcast((128, D_OUT, N_OUT))
                v3 = vd[:].rearrange("p (d j) -> p d j", d=D_OUT, j=N_OUT)
                nc.vector.tensor_tensor(v3, s3, f3, AluOp.mult)
            else:
                # final output, reference layout v[b, j*32+d]
                vf = v_fin[:].rearrange("p (j d) -> p d j", j=N_OUT, d=D_OUT)
                nc.vector.tensor_tensor(
                    vf,
                    s_rep[0:BC, :].rearrange(
                        "p (d j) -> p d j", d=D_OUT, j=N_OUT),
                    fsc[0:BC, :].unsqueeze(1).to_broadcast(
                        (BC, D_OUT, N_OUT)),
                    AluOp.mult)
                nc.gpsimd.dma_start(v_out[q * BC:(q + 1) * BC, :], v_fin[:])

        def rounds(q):
            """Rounds t=1,2 for chunk q; the t=1 agreement was already
            emitted inline with phase 1, and the final squash is deferred to
            the caller (hides the last AllReduce's latency)."""
            ar_out = None
            for t in (1, 2):
                if t == 2:
                    for sec in range(NSEC):
                        agreement_section(t, sec)
                softmax()
                ar_out = weighted_sum(q, t)
                if t < 2:
                    squash(ar_out, 0, False, q)
            return ar_out

        # ------------------------------------------------------ emission
        s0_burst()
        nc.sync.dma_start(xz[:], xz_in[:])   # first needed by phase1 blk 0
        nc.scalar.copy(s0_sb[:], ps_s0[:])
        nc.gpsimd.dma_start(ar0_in[:], s0_sb[:])
        nc.gpsimd.collective_compute(
            "AllReduce", AluOp.add, replica_groups=rg,
            ins=[ar0_in.opt()], outs=[ar0_out.opt()],
        )
        # Both chunks' v0-squashes are emitted inside chunk-0 phase 1 right
        # before the first inlined agreement section: the DVE drains early
        # psums while the s0 AllReduce is in flight, and emitting the
        # chunk-1 squash HERE (before any later collective exists) avoids
        # the dep-tracker's monotonic collective-completion wait.
        def pre0():
            squash(ar0_out, 0, False, 0, dma_eng="g")
        phase1(0, agree_lag=3,
               pre_agree=pre0, dve_drain_mod=1)
        ar_last0 = rounds(0)
        squash(ar0_out, BC, False, 1)       # v0 for chunk 1
        # chunk 1 phase 1 overlaps chunk 0's final AllReduce latency
        phase1(1, agree_lag=0)
        squash(ar_last0, 0, True, 0)        # chunk 0 output
        ar_last1 = rounds(1)
        squash(ar_last1, 0, True, 1)

    _fix_sync_waits(nc)
    return nc


# ---------------------------------------------------------------- host prep
def _prep_inputs(x, W):
    """Per-core input maps.

    Local capsule l = blk*8 + g2*4 + gp*2 + h.
    SBUF rows r128 = g2*64 + gp*32 + hp*16 + k.
      wt[r128; blk*1024 + d*32 + j] = W[l(blk,g2,gp,hp), j, d, k]
      xz[r128; blk*128 + h*64 + b]  = x[b, l(blk,g2,gp,h), k] if hp==h else 0
      xd[r128; blk*64 + b]          = x[b, l(blk,g2,gp,hp), k] / 32
    """
    import jax.numpy as jnp

    def tobf(a):
        return np.asarray(jnp.asarray(a).astype(jnp.bfloat16))

    in_maps = []
    ones32 = np.zeros((128, BC), np.float32)
    for p in range(128):
        ones32[p, p % 32] = 1.0
    ones32 = tobf(ones32)
    for c in range(N_CORES):
        xi = x[:, c * I_LOC:(c + 1) * I_LOC, :]          # [B, 256, 16]
        wi = W[0, c * I_LOC:(c + 1) * I_LOC]             # [256, 32, 32, 16]
        # l = blk*8 + g2*4 + gp*2 + h
        x6 = xi.reshape(B, NBLK, 2, 2, 2, D_IN)          # b,blk,g2,gp,h,k
        w7 = wi.reshape(NBLK, 2, 2, 2, N_OUT, D_OUT, D_IN)  # blk,g2,gp,h,j,d,k

        # wt[(g2,gp,hp,k); (blk, d, j)]
        wt = np.transpose(w7, (1, 2, 3, 6, 0, 5, 4)).reshape(128, NBLK * JD)

        # xz[(g2,gp,hp,k); (blk, h, b)] with h-select zero interleave
        xt = np.transpose(x6, (2, 3, 4, 5, 1, 0))        # g2,gp,h,k,blk,b
        xz = np.zeros((2, 2, 2, D_IN, NBLK, 2, B), np.float32)
        for h in range(2):
            xz[:, :, h, :, :, h, :] = xt[:, :, h]
        xz = xz.reshape(128, NBLK * 2 * B)

        # xd[(g2,gp,hp,k); (blk, b)] = x/32 dense
        xd = (xt / 32.0).reshape(128, NBLK * B)

        in_maps.append({
            "wt": tobf(np.ascontiguousarray(wt)),
            "xz": tobf(np.ascontiguousarray(xz)),
            "xd": tobf(np.ascontiguousarray(xd)),
            "ones32": ones32,
        })
    return in_maps


_cached = {}


def _get_program():
    if "nc" not in _cached:
        _cached["nc"] = _build_program()
    return _cached["nc"]


def kernel(x, W):
    x = np.asarray(x, dtype=np.float32)
    W = np.asarray(W, dtype=np.float32)
    nc = _get_program()
    in_maps = _prep_inputs(x, W)
    res = bass_utils.run_bass_kernel_spmd(
        nc, in_maps, core_ids=list(range(N_CORES))
    )
    v = res.results[0]["v"].reshape(B, N_OUT, D_OUT)
    return v.astype(np.float32)



# revision 7
# speedup vs baseline: 1.1907x; 1.1907x over previous
"""Trainium2 Bass kernel for CapsuleLayer dynamic routing (v5).

Problem: x [64, 2048, 16], W [1, 2048, 32, 32, 16] ->
  u_hat = einsum('bik,ijdk->bijd', x, W[0])           [B, N_in, N_out, D_out]
  3 rounds of routing (softmax over j, weighted sum over i, squash),
  returns v [64, 32, 32].

Sharding: N_in (2048) split over 8 cores, 256 local capsules each; per-round
partial weighted sums AllReduced; softmax/squash replicated.

v5 redesign (vs the v2 baseline at 745us):
  * Batch in 4 chunks of 16; chunk round-chains are interleaved two at a
    time so every AllReduce's latency hides under the other chunk's DVE
    work (U double-buffered at 64KB/partition per chunk).
  * phase 1 emits u_hat with M=128 PSUM rows ((e8,b16) out partitions,
    K=(e8,k16)=128 zero-interleaved x stationary, W streamed as the
    moving operand): 4x less PE time than the 32-row-tile baseline.
  * The agreement's reduce over d is NOT a DVE tree: the PE accumulates
    the 32 d-slices of the product P=U*v into logits PSUM through an
    identity stationary (f32 accumulation, frees ~130us of DVE).
  * Weighted-sum keeps the ones-matmul i-reduction; products (U*v, U*c)
    are the only big DVE work left, at 2x_1p bf16.
  * All PSUM->SBUF drains on ACT; AllReduce machinery + replicate DMAs
    on the Pool queue (interleaved in dependency order); W streamed once
    per chunk on the sync queue.
"""
import sys

sys.path.insert(0, '/opt/trn_rl_repo')

import numpy as np

import concourse.bass as bass
import concourse.mybir as mybir
from concourse import bass_utils, tile

# ---------------------------------------------------------------- constants
N_CORES = 8
B = 64
N_IN = 2048
D_IN = 16
N_OUT = 32
D_OUT = 32
EPS = 1e-9

I_LOC = N_IN // N_CORES          # 256 local capsules
NG = 32                          # capsule groups of 8 (i = g*8 + e)
BC = 16                          # batch chunk
NCHUNK = B // BC                 # 4
JD = N_OUT * D_OUT               # 1024 (d,j) values per capsule
UCOLS = NG * JD                  # 32768 U columns per chunk
SECG = 4                         # capsule groups per section
NSEC = NG // SECG                # 8 sections per chunk
SEC_COLS = SECG * JD             # 4096

f32 = mybir.dt.float32
bf16 = mybir.dt.bfloat16

_MAX_WAITS = 1
_carrier = [0]


def _patch_tile():
    """Work around this walrus build rejecting >1 sync wait per instruction."""
    import concourse.mybir as _mybir
    from concourse import tile as _tile
    from concourse.tile import TileContext as _TC

    def _drain_and_barrier(self, tick_clock, wait_clock):
        ScopedClock = _tile.ScopedClock
        probe = self.nc.sync.nop(nofuse=True)
        wait_clock.add_sem_waits(
            probe.ins, ScopedClock({None: tick_clock.global_clock})
        )
        si = probe.ins.sync_info
        waits = list(si.on_wait)
        probe.ins.sync_info = _mybir.SyncInfo(
            on_wait=waits[:1], on_update=list(si.on_update)
        )
        for w in waits[1:]:
            carrier = self.nc.sync.nop(nofuse=True)
            carrier.ins.sync_info = _mybir.SyncInfo(on_wait=[w], on_update=[])
        self.nc.sync.drain()
        self.nc.all_engine_barrier()
        assert self.sems is not None
        popped = self.nc._tile_sem_poison_stack.pop()
        assert popped is self._sem_poison
        self.nc.clear_and_free_semaphores(list(self.sems.allocated().values()))
        self.nc.all_engine_barrier()

    _TC._drain_and_barrier = _drain_and_barrier

    try:
        from concourse import tile_utils
        tile_utils.max_sbuf_usage = 208 * 1024
    except Exception:
        pass


def _fix_sync_waits(nc, max_waits=_MAX_WAITS):
    n_fixed = 0
    for func in nc.m.functions:
        for bb in func.blocks:
            insts = list(bb.instructions)
            new_list = []
            changed = False
            for inst in insts:
                si = getattr(inst, "sync_info", None)
                waits = list(si.on_wait) if si is not None else []
                if len(waits) > max_waits:
                    keep = waits[: max_waits - 1] if max_waits > 1 else []
                    hoist = waits[len(keep):-1]
                    tail = [waits[-1]]
                    for w in hoist:
                        _carrier[0] += 1
                        nop = mybir.InstNoOp(
                            name=f"syncfix-{_carrier[0]}", engine=inst.engine
                        )
                        nop.sync_info = mybir.SyncInfo(on_wait=[w], on_update=[])
                        new_list.append(nop)
                    inst.sync_info = mybir.SyncInfo(
                        on_wait=keep + tail, on_update=list(si.on_update)
                    )
                    changed = True
                    n_fixed += 1
                new_list.append(inst)
            if changed:
                bb.instructions = new_list
    return n_fixed


# ---------------------------------------------------------------- program
def _build_program():
    _patch_tile()
    nc = bass.Bass(trn_type="TRN2", num_devices=N_CORES)

    wt_in = nc.dram_tensor("wt", [128, UCOLS], bf16, kind="ExternalInput")
    xin_in = nc.dram_tensor("xin", [128, NCHUNK * NG * 128], bf16,
                            kind="ExternalInput")
    xd_in = nc.dram_tensor("xd", [128, NG * B], bf16, kind="ExternalInput")
    id_in = nc.dram_tensor("ident", [128, 128], bf16, kind="ExternalInput")
    ones_in = nc.dram_tensor("ones16", [128, BC], bf16, kind="ExternalInput")
    v_out = nc.dram_tensor("v", [B, JD], f32, kind="ExternalOutput")

    AluOp = mybir.AluOpType
    Act = mybir.ActivationFunctionType
    Axis = mybir.AxisListType
    rg = [list(range(N_CORES))]

    from contextlib import ExitStack
    with tile.TileContext(nc, num_cores=N_CORES) as tc, ExitStack() as es:
        cpool = es.enter_context(tc.tile_pool(name="const", bufs=1))
        wpool = es.enter_context(tc.tile_pool(name="wstream", bufs=4))
        upool = es.enter_context(tc.tile_pool(name="ubuf", bufs=1))
        scpool = es.enter_context(tc.tile_pool(name="scratch", bufs=1))
        smpool = es.enter_context(tc.tile_pool(name="small", bufs=1))
        psph = es.enter_context(tc.tile_pool(name="psph1", bufs=2, space="PSUM"))
        pslg = es.enter_context(tc.tile_pool(name="pslog", bufs=1, space="PSUM"))
        psws = es.enter_context(tc.tile_pool(name="psws", bufs=1, space="PSUM"))
        dpool = es.enter_context(tc.tile_pool(name="dram", bufs=1, space="DRAM"))

        # ---- constants / inputs resident in SBUF
        ident = cpool.tile([128, 128], bf16, tag="ident")
        ones16 = cpool.tile([128, BC], bf16, tag="ones16")
        xd = cpool.tile([128, NG * B], bf16, tag="xd")
        nc.sync.dma_start(ident[:], id_in[:])
        nc.sync.dma_start(ones16[:], ones_in[:])
        nc.sync.dma_start(xd[:], xd_in[:])

        # ---- big buffers
        U = [upool.tile([128, UCOLS], bf16, tag=f"U{h}", name=f"U{h}")
             for h in range(2)]
        xint = [cpool.tile([128, NG * 128], bf16, tag=f"xint{h}",
                           name=f"xint{h}") for h in range(2)]
        slots = [scpool.tile([128, SEC_COLS], bf16, tag=f"slot{h}",
                             name=f"slot{h}") for h in range(2)]
        algA = [smpool.tile([128, NG * N_OUT], f32, tag=f"algA{h}",
                            name=f"algA{h}") for h in range(2)]
        cbuf = [smpool.tile([128, NG * N_OUT], bf16, tag=f"c{h}",
                            name=f"c{h}") for h in range(2)]
        vt = [smpool.tile([128, JD], bf16, tag=f"vt{h}", name=f"vt{h}")
              for h in range(2)]
        ebuf = smpool.tile([128, NG * N_OUT], f32, tag="ebuf")
        v0all = smpool.tile([128, JD], bf16, tag="v0all")
        s_rep = smpool.tile([128, JD], f32, tag="srep")
        Zt = smpool.tile([128, NG], f32, tag="Zt")
        Zr = smpool.tile([128, NG], f32, tag="Zr")
        # aliases: squash's square scratch reuses ebuf; the softmax
        # Zr-replica reuses s_rep (lifetimes strictly serialized on DVE).
        p2 = ebuf
        zrep = s_rep
        s2 = smpool.tile([128, N_OUT], f32, tag="s2")
        lns = smpool.tile([128, N_OUT], f32, tag="lns")
        rsq = smpool.tile([128, N_OUT], f32, tag="rsq")
        den = smpool.tile([128, N_OUT], f32, tag="den")
        rinv = smpool.tile([128, N_OUT], f32, tag="rinv")
        fsc = smpool.tile([128, N_OUT], f32, tag="fsc")
        ibuf = smpool.tile([128, N_OUT], mybir.dt.int32, tag="ibuf")
        s_sb = smpool.tile([B, JD], f32, tag="ssb")
        v_fin = smpool.tile([BC, JD], f32, tag="vfin")

        # PSUM tiles
        ps_log = pslg.tile([128, NG * N_OUT], f32, tag="pslog")
        ps_ws = psws.tile([B, JD], f32, tag="psws")

        # AR dram staging
        ar0_in = dpool.tile([B, JD], f32, tag="ar0i")
        ar0_out = dpool.tile([B, JD], f32, tag="ar0o")
        ar_bufs = {}
        for q in range(NCHUNK):
            for t in (1, 2):
                ar_bufs[(q, t)] = (
                    dpool.tile([BC, JD], f32, name=f"ari{q}{t}"),
                    dpool.tile([BC, JD], f32, name=f"aro{q}{t}"),
                )

        # ------------------------------------------------------ phase 1
        def warmup():
            """Back-to-back full matmuls to ramp the PE p-state before the
            s0/phase-1 burst (output never read)."""
            ps = psph.tile([128, JD], f32, tag="ph1", name="warm")
            for i in range(10):
                nc.tensor.matmul(
                    ps[:, 0:512], xd[:, 0:128], xd[:, 0:512],
                    start=True, stop=True,
                )

        def phase1(q, g0=0, g1=NG, with_s0=False):
            """u_hat groups [g0,g1) for batch chunk q into U[q%2]; W
            streamed per group on the sync queue; PSUM drained on ACT.
            If with_s0, also accumulates s0 = sum_i u_hat/32 for the full
            batch into ps_ws through the dense xd stationary.  Mid-kernel
            chunks are emitted in two halves around the concurrent round's
            softmax so the ACT/PE FIFOs never head-of-line-block it."""
            Uq = U[q % 2]
            xq = xint[q % 2]
            if g0 == 0:
                nc.sync.dma_start(
                    xq[:], xin_in[:, q * NG * 128:(q + 1) * NG * 128])
            for g in range(g0, g1):
                w = wpool.tile([128, JD], bf16, tag="w")
                nc.sync.dma_start(w[:], wt_in[:, g * JD:(g + 1) * JD])
                ps = psph.tile([128, JD], f32, tag="ph1")
                for half in range(2):
                    if with_s0:
                        nc.tensor.matmul(
                            ps_ws[:, half * 512:(half + 1) * 512],
                            xd[:, g * B:(g + 1) * B],
                            w[:, half * 512:(half + 1) * 512],
                            start=(g == 0), stop=(g == NG - 1),
                        )
                    nc.tensor.matmul(
                        ps[:, half * 512:(half + 1) * 512],
                        xq[:, g * 128:(g + 1) * 128],
                        w[:, half * 512:(half + 1) * 512],
                        start=True, stop=True,
                    )
                nc.scalar.copy(Uq[:, g * JD:(g + 1) * JD], ps[:])

        # ------------------------------------------------------ routing ops
        def agreement(q, t):
            """logits psum[p=(e,b), (g,j)] = sum_d U*v via DVE product +
            PE identity-matmul accumulation over the 32 d slices."""
            Uq = U[q % 2]
            v4 = (vt[q % 2][:]
                  .rearrange("p (d j) -> p d j", d=D_OUT, j=N_OUT)
                  .unsqueeze(1)
                  .to_broadcast((128, SECG, D_OUT, N_OUT)))
            for sec in range(NSEC):
                slot = slots[sec % 2]
                P = slot[:].rearrange(
                    "p (g d j) -> p g d j", g=SECG, d=D_OUT, j=N_OUT)
                Us = Uq[:, sec * SEC_COLS:(sec + 1) * SEC_COLS].rearrange(
                    "p (g d j) -> p g d j", g=SECG, d=D_OUT, j=N_OUT)
                nc.vector.tensor_tensor(P, Us, v4, AluOp.mult)   # 2x
                for dd in range(D_OUT):
                    nc.tensor.matmul(
                        ps_log[:, sec * SECG * N_OUT:
                               (sec + 1) * SECG * N_OUT],
                        ident[:],
                        P[:, :, dd, :],
                        start=(dd == 0), stop=(dd == D_OUT - 1),
                    )

        def softmax(q, t):
            """c = softmax over j of logits (+ prev-round logits for t=2)."""
            A = algA[q % 2]
            if t == 1:
                nc.scalar.copy(A[:], ps_log[:])
            else:
                nc.vector.tensor_add(A[:], A[:], ps_log[:])
            nc.scalar.activation(ebuf[:], A[:], Act.Exp)
            e3 = ebuf[:].rearrange("p (g j) -> p g j", g=NG, j=N_OUT)
            nc.vector.reduce_sum(Zt[:], e3, axis=Axis.X)
            nc.vector.reciprocal(Zr[:], Zt[:])
            nc.vector.tensor_copy(
                zrep[:].rearrange("p (g j) -> p g j", g=NG, j=N_OUT),
                Zr[:].unsqueeze(2).to_broadcast((128, NG, N_OUT)))
            nc.vector.tensor_tensor(
                cbuf[q % 2][:], ebuf[:], zrep[:], AluOp.mult)

        def weighted_sum(q, t):
            """s_partial[b,(d,j)] = sum_i c*U: DVE product (2x) + PE
            ones-matmul reduction over (e-partitions, g-psum-accum)."""
            Uq = U[q % 2]
            cq = cbuf[q % 2]
            for sec in range(NSEC):
                slot = slots[sec % 2]
                P = slot[:].rearrange(
                    "p (g d j) -> p g d j", g=SECG, d=D_OUT, j=N_OUT)
                Us = Uq[:, sec * SEC_COLS:(sec + 1) * SEC_COLS].rearrange(
                    "p (g d j) -> p g d j", g=SECG, d=D_OUT, j=N_OUT)
                c4 = (cq[:, sec * SECG * N_OUT:(sec + 1) * SECG * N_OUT]
                      .rearrange("p (g j) -> p g j", g=SECG, j=N_OUT)
                      .unsqueeze(2)
                      .to_broadcast((128, SECG, D_OUT, N_OUT)))
                nc.vector.tensor_tensor(P, Us, c4, AluOp.mult)   # 2x
                for g in range(SECG):
                    for half in range(2):
                        nc.tensor.matmul(
                            ps_ws[0:BC, half * 512:(half + 1) * 512],
                            ones16[:],
                            slot[:, g * JD + half * 512:
                                 g * JD + half * 512 + 512],
                            start=(sec == 0 and g == 0),
                            stop=(sec == NSEC - 1 and g == SECG - 1),
                        )
            nc.scalar.copy(s_sb[0:BC, :], ps_ws[0:BC, :])
            ar_in, ar_out = ar_bufs[(q, t)]
            nc.gpsimd.dma_start(ar_in[:], s_sb[0:BC, :])
            nc.gpsimd.collective_compute(
                "AllReduce", AluOp.add, replica_groups=rg,
                ins=[ar_in.opt()], outs=[ar_out.opt()],
            )
            return ar_out

        def squash_core(rows, out_tt):
            """Common squash tail: rows = partition count holding s in
            s_rep; out_tt(s3, f3) emits the final multiply."""
            nc.scalar.square(p2[0:rows, :], s_rep[0:rows, :])
            p3 = p2[0:rows, :].rearrange("p (d j) -> p j d", d=D_OUT, j=N_OUT)
            nc.vector.reduce_sum(s2[0:rows, :], p3, axis=Axis.X)
            nc.vector.tensor_scalar_add(den[0:rows, :], s2[0:rows, :],
                                        1.0 + EPS)
            nc.vector.tensor_scalar_add(lns[0:rows, :], s2[0:rows, :], EPS)
            ii = lns[0:rows, :].bitcast(mybir.dt.int32)
            nc.vector.tensor_scalar(
                ibuf[0:rows, :], ii, 1, None,
                mybir.AluOpType.logical_shift_right)
            nc.vector.tensor_scalar(
                ibuf[0:rows, :], ibuf[0:rows, :], 0x5F3759DF, -1,
                mybir.AluOpType.subtract, mybir.AluOpType.mult)
            y0 = ibuf[0:rows, :].bitcast(f32)
            nc.vector.tensor_mul(rsq[0:rows, :], y0, y0)
            nc.vector.tensor_mul(rsq[0:rows, :], rsq[0:rows, :],
                                 lns[0:rows, :])
            nc.vector.tensor_scalar(
                rsq[0:rows, :], rsq[0:rows, :], -0.5, 1.5,
                mybir.AluOpType.mult, mybir.AluOpType.add)
            nc.vector.tensor_mul(rsq[0:rows, :], rsq[0:rows, :], y0)
            nc.vector.tensor_mul(fsc[0:rows, :], rsq[0:rows, :],
                                 rsq[0:rows, :])
            nc.vector.tensor_mul(fsc[0:rows, :], fsc[0:rows, :],
                                 lns[0:rows, :])
            nc.vector.tensor_scalar(
                fsc[0:rows, :], fsc[0:rows, :], -0.5, 1.5,
                mybir.AluOpType.mult, mybir.AluOpType.add)
            nc.vector.tensor_mul(rsq[0:rows, :], rsq[0:rows, :],
                                 fsc[0:rows, :])
            nc.vector.reciprocal(rinv[0:rows, :], den[0:rows, :])
            nc.vector.tensor_mul(fsc[0:rows, :], rsq[0:rows, :],
                                 rinv[0:rows, :])
            nc.vector.tensor_mul(fsc[0:rows, :], fsc[0:rows, :],
                                 s2[0:rows, :])
            out_tt()

        def squash_v0():
            """v0 = squash(AllReduce(s0)) for the full batch at
            [p=(rep2,b64)], then per-chunk replicated tiles for c0/c1."""
            for r in range(2):
                nc.gpsimd.dma_start(s_rep[64 * r:64 * r + 64, :], ar0_out[:])

            def tt():
                s3 = s_rep[:].rearrange("p (d j) -> p d j", d=D_OUT, j=N_OUT)
                f3 = fsc[:].unsqueeze(1).to_broadcast((128, D_OUT, N_OUT))
                v3 = v0all[:].rearrange("p (d j) -> p d j", d=D_OUT, j=N_OUT)
                nc.vector.tensor_tensor(v3, s3, f3, AluOp.mult)
            squash_core(128, tt)

        def rep_v0(q):
            """vt[q%2] <- per-chunk (e8,b16)-replicated slice of v0all."""
            dst = vt[q % 2]
            for e in range(8):
                nc.gpsimd.dma_start(
                    dst[16 * e:16 * e + 16, :],
                    v0all[q * BC:(q + 1) * BC, :])

        def squash_round(q, ar_out):
            """v_{t} for chunk q from its AllReduced s, into vt[q%2]."""
            for e in range(8):
                nc.gpsimd.dma_start(
                    s_rep[16 * e:16 * e + 16, :], ar_out[:])

            def tt():
                s3 = s_rep[:].rearrange("p (d j) -> p d j", d=D_OUT, j=N_OUT)
                f3 = fsc[:].unsqueeze(1).to_broadcast((128, D_OUT, N_OUT))
                v3 = vt[q % 2][:].rearrange(
                    "p (d j) -> p d j", d=D_OUT, j=N_OUT)
                nc.vector.tensor_tensor(v3, s3, f3, AluOp.mult)
            squash_core(128, tt)

        def squash_final(q, ar_out):
            """Final v for chunk q -> v_out rows, reference layout."""
            for e in range(8):
                nc.gpsimd.dma_start(
                    s_rep[16 * e:16 * e + 16, :], ar_out[:])

            def tt():
                vf = v_fin[:].rearrange("p (j d) -> p d j", j=N_OUT, d=D_OUT)
                nc.vector.tensor_tensor(
                    vf,
                    s_rep[0:BC, :].rearrange(
                        "p (d j) -> p d j", d=D_OUT, j=N_OUT),
                    fsc[0:BC, :].unsqueeze(1).to_broadcast(
                        (BC, D_OUT, N_OUT)),
                    AluOp.mult)
                nc.gpsimd.dma_start(v_out[q * BC:(q + 1) * BC, :], v_fin[:])
            squash_core(BC, tt)

        def round_(q, t):
            agreement(q, t)
            softmax(q, t)
            return weighted_sum(q, t)

        # ------------------------------------------------------ emission
        # S0: warm PE, stream W once for chunk 0 while accumulating s0 for
        # the full batch; AllReduce s0; squash v0.
        warmup()
        phase1(0, with_s0=True)
        nc.scalar.copy(s_sb[:], ps_ws[:])
        nc.gpsimd.dma_start(ar0_in[:], s_sb[:])
        nc.gpsimd.collective_compute(
            "AllReduce", AluOp.add, replica_groups=rg,
            ins=[ar0_in.opt()], outs=[ar0_out.opt()],
        )
        squash_v0()
        rep_v0(0)
        rep_v0(1)

        # S1: t1c0 (phase1 c1 interleaved in halves around the softmax)
        phase1(1, 0, 16)
        agreement(0, 1)
        softmax(0, 1)
        phase1(1, 16, NG)
        ar_c0t1 = weighted_sum(0, 1)
        # S2: t1c1
        ar_c1t1 = round_(1, 1)
        # S3: t2c0
        squash_round(0, ar_c0t1)
        ar_c0t2 = round_(0, 2)
        # S4: t2c1 (phase1 c2 overlaps; U0 free after t2c0)
        squash_round(1, ar_c1t1)
        rep_v0(2)
        phase1(2, 0, 16)
        agreement(1, 2)
        softmax(1, 2)
        phase1(2, 16, NG)
        ar_c1t2 = weighted_sum(1, 2)
        # S5: t1c2 (phase1 c3 overlaps; U1 free after t2c1)
        squash_final(0, ar_c0t2)
        rep_v0(3)
        phase1(3, 0, 16)
        agreement(2, 1)
        softmax(2, 1)
        phase1(3, 16, NG)
        ar_c2t1 = weighted_sum(2, 1)
        # S6: t1c3
        squash_final(1, ar_c1t2)
        ar_c3t1 = round_(3, 1)
        # S7: t2c2
        squash_round(2, ar_c2t1)
        ar_c2t2 = round_(2, 2)
        # S8: t2c3
        squash_round(3, ar_c3t1)
        ar_c3t2 = round_(3, 2)
        # tail
        squash_final(2, ar_c2t2)
        squash_final(3, ar_c3t2)

    _fix_sync_waits(nc)
    return nc


# ---------------------------------------------------------------- host prep
def _prep_inputs(x, W):
    """Per-core input maps.

    Local capsule l = g*8 + e (g in [0,32), e in [0,8)).
    SBUF rows r128 = e*16 + k.
      wt[(e,k); g*1024 + d*32 + j]         = W[l(g,e), j, d, k]
      xin[(e',k); c*4096 + g*128 + e*16+bb] = [e==e'] x[c*16+bb, l(g,e), k]
      xd[(e,k); g*64 + b]                  = x[b, l(g,e), k] / 32
    """
    import jax.numpy as jnp

    def tobf(a):
        return np.asarray(jnp.asarray(a).astype(jnp.bfloat16))

    in_maps = []
    ident = tobf(np.eye(128, dtype=np.float32))
    ones16 = np.zeros((128, BC), np.float32)
    for p in range(128):
        ones16[p, p % BC] = 1.0
    ones16 = tobf(ones16)
    for c in range(N_CORES):
        xi = np.asarray(x[:, c * I_LOC:(c + 1) * I_LOC, :])   # [B, 256, 16]
        wi = np.asarray(W[0, c * I_LOC:(c + 1) * I_LOC])      # [256, 32, 32, 16]

        w5 = wi.reshape(NG, 8, N_OUT, D_OUT, D_IN)            # g,e,j,d,k
        wt = np.transpose(w5, (1, 4, 0, 3, 2)).reshape(128, UCOLS)

        x5 = xi.reshape(NCHUNK, BC, NG, 8, D_IN)              # c,bb,g,e,k
        xin = np.zeros((8, D_IN, NCHUNK, NG, 8, BC), np.float32)
        for e in range(8):
            xin[e, :, :, :, e, :] = np.transpose(
                x5[:, :, :, e, :], (3, 0, 2, 1))
        xin = xin.reshape(128, NCHUNK * NG * 128)

        xd = (np.transpose(xi.reshape(B, NG, 8, D_IN),
                           (2, 3, 1, 0)) / 32.0).reshape(128, NG * B)

        in_maps.append({
            "wt": tobf(np.ascontiguousarray(wt)),
            "xin": tobf(np.ascontiguousarray(xin)),
            "xd": tobf(np.ascontiguousarray(xd)),
            "ident": ident,
            "ones16": ones16,
        })
    return in_maps


_cached = {}


def _get_program():
    if "nc" not in _cached:
        _cached["nc"] = _build_program()
    return _cached["nc"]


def kernel(x, W):
    x = np.asarray(x, dtype=np.float32)
    W = np.asarray(W, dtype=np.float32)
    nc = _get_program()
    in_maps = _prep_inputs(x, W)
    res = bass_utils.run_bass_kernel_spmd(
        nc, in_maps, core_ids=list(range(N_CORES))
    )
    v = res.results[0]["v"].reshape(B, N_OUT, D_OUT)
    return v.astype(np.float32)
